# revision 11
# baseline (speedup 1.0000x reference)
"""Pointer-generator head on 8 Trainium2 NeuronCores (Bass/Tile).

Computation (per batch row b):
    p_gen = sigmoid(context @ w_c + state @ w_s + emb @ w_y + b)
    out   = p_gen * vocab_dist
    out[b, src_ids[b, t]] += (1 - p_gen) * attn_dist[b, t]   (masked, clamped)

Sharding: batch dim (512) split across 8 cores, 64 rows each; every core keeps
its rows' full V=32000 columns so the scatter-add stays core-local; the small
parameter vectors are replicated.

Per-core device kernel (interleaved layout: partition p = 2b+h holds row b's
half-row h = [h*16000, (h+1)*16000) contiguously):

  * p_gen: 20 accumulating fp16 PE matmuls produce per-row dots [64,1] in f32
    PSUM; a 0/1 duplication matmul expands them to the interleaved [128,1]
    layout; sigmoid on the scalar engine.
  * scatter prep: the host buckets the T=400 items per row by 2000-column
    target bucket (8 per partition), laying each duplicate-id group out as
    consecutive slots.  The device computes group sums with a single DVE
    prefix scan (state = cont*state + attn, f32 state): each group's last
    slot holds its total.  Group sums are scaled by (1 - p_gen), in bf16.
  * stream: 8 chunks of [128, 2000] f32.  Per chunk, one GPSIMD local_scatter
    writes the chunk's bf16 group sums into a bf16 sparse tile (slots that
    are not a group's last member carry index -1 and are dropped); one DVE
    scalar_tensor_tensor computes p_gen * dense + sparse, emitting bf16; the
    result streams back out as bf16 and the host widens it to f32 (pure
    dtype relayout).  The packed sideband load and the dense input stream own
    the SP queue (sideband first); output DMAs use the scalar-engine queue.

Sideband metadata is bit-packed by the host into two [128, *] tensors (fp16
x^T + weights for the p_gen path; bf16 attn slots + bf16 continuation flags +
int16 scatter indices for the scatter path) so two DMA configs cover it, both
issued ahead of the dense stream.  Host-side work is limited to index metadata
and pure data relayout (sharding, transposition, dtype casts).
"""

import os

import ml_dtypes
import numpy as np

import concourse.bacc as bacc
import concourse.mybir as mybir
import concourse.tile as tile
from concourse import bass_utils

# ---- problem shape (hardcoded per spec) ----
B = 512
T = 400
V = 32000
ENC, HID, EMB = 1024, 1024, 512
NCORES = 8

P = 128
BSH = B // NCORES       # 64 rows per core
HV = V // 2             # half-row width per partition
D = ENC + HID + EMB     # 2560
NK = D // P             # K-chunks for the p_gen matmul
NB = 8                  # scatter buckets per partition
BW = HV // NB           # 2000 f32 columns per bucket
NSTREAM = 8
SW = HV // NSTREAM      # 2000 f32 per partition per stream chunk (== BW)
XW = NK * BSH           # 1280 fp16 x^T columns per partition

F32 = mybir.dt.float32
F16 = mybir.dt.float16
BF16 = mybir.dt.bfloat16
I16 = mybir.dt.int16

NPBF16 = ml_dtypes.bfloat16


# --------------------------------------------------------------------------
# host-side index prep (pure metadata / relayout)
# --------------------------------------------------------------------------

def _bucketize(src_ids: np.ndarray, vocab_size: int):
    """Bucket one shard's unmasked items by (partition, bucket).

    Returns buckets[p][c] = list of (bucket-local target, [t indices]) groups;
    each group shares one raw id.  Masked items (id >= min(vocab_size, V))
    contribute nothing and are dropped.
    """
    id_lim = min(int(vocab_size), V)
    buckets = [[[] for _ in range(NB)] for _ in range(P)]
    for b in range(BSH):
        order: dict[int, list[int]] = {}
        for t, i in enumerate(src_ids[b].tolist()):
            if i < id_lim:
                order.setdefault(i, []).append(t)
        for i, g in order.items():
            h, off = divmod(i, HV)
            c = off // BW
            buckets[2 * b + h][c].append((off - c * BW, g))
    return buckets


def _prep_shard(attn: np.ndarray, src_ids: np.ndarray, vocab_size: int, S: int):
    """-> (attn_p, cont, lsidx): [P, NB*S] bf16, [P, NB*S] bf16, [P, NB*S] i16.

    Slot layout per partition: NB segments of S slots; every duplicate-id
    group occupies consecutive slots.  cont is 0 on each group's first slot
    and 1 on the rest, so the device prefix scan state = cont*state + attn
    leaves the group total on its LAST slot; lsidx carries the group's
    bucket-local f32 target column on that last slot and -1 (dropped)
    everywhere else.  Pad slots have attn 0, cont 0, lsidx -1."""
    TS = NB * S
    attn_f = np.zeros((P, TS), np.float32)
    cont_f = np.zeros((P, TS), np.float32)
    lsidx = np.full((P, TS), -1, np.int16)
    buckets = _bucketize(src_ids, vocab_size)
    for p in range(P):
        row = p // 2
        for c in range(NB):
            j = c * S
            for loc, ts in buckets[p][c]:
                for k, t in enumerate(ts):
                    attn_f[p, j + k] = attn[row, t]
                    cont_f[p, j + k] = 0.0 if k == 0 else 1.0
                j += len(ts)
                lsidx[p, j - 1] = loc
            assert j <= (c + 1) * S
    return attn_f.astype(NPBF16), cont_f.astype(NPBF16), lsidx


def _slot_requirement(src_ids_full: np.ndarray, vocab_size: int):
    """Global S: max items in any (core, partition, bucket), rounded even."""
    mx = 2
    for c in range(NCORES):
        buckets = _bucketize(src_ids_full[c * BSH : (c + 1) * BSH], vocab_size)
        for p in range(P):
            for ch in range(NB):
                n = sum(len(ts) for _, ts in buckets[p][ch])
                mx = max(mx, n)
    return (mx + 1) // 2 * 2


# --------------------------------------------------------------------------
# device kernel (per core; SPMD across 8 cores)
# --------------------------------------------------------------------------

def _build_kernel(tc: tile.TileContext, out, ins, b_const: float, S: int):
    nc = tc.nc
    vd, xwside, scside, dup = ins
    TS = NB * S
    # packed sideband column offsets (int16 units)
    AT0, CT0, LS0 = 0, TS, 2 * TS

    with tc.tile_pool(name="small", bufs=1) as sp, \
         tc.tile_pool(name="psum", bufs=1, space="PSUM") as pp, \
         tc.tile_pool(name="stream", bufs=8) as pool, \
         tc.tile_pool(name="sparse", bufs=6) as spp:
        # ---- sidebands first on the SP queue: p_gen inputs, scatter pack ----
        xw = sp.tile([P, XW + NK], F16)
        nc.sync.dma_start(xw[:], xwside[:, :])
        sb = sp.tile([P, 3 * TS], I16)
        nc.sync.dma_start(sb[:], scside[:, :])
        dupt = sp.tile([BSH, P], F16)
        nc.sync.dma_start(dupt[:], dup[:, :])

        # ---- dense input stream: 8 chunk loads follow on the SP queue ----
        vdv = vd.rearrange("(p v) -> p v", p=P)
        outv = out.rearrange("(p v) -> p v", p=P)
        tls = []
        for c in range(NSTREAM):
            tl = pool.tile([P, SW], F32, tag="stream")
            nc.sync.dma_start(tl[:], vdv[:, c * SW : (c + 1) * SW])
            tls.append(tl)
        xt = xw[:, :XW]
        wt = xw[:, XW : XW + NK]
        at = sb[:, AT0 : AT0 + TS].bitcast(BF16)
        ct = sb[:, CT0 : CT0 + TS].bitcast(BF16)
        lsi = sb[:, LS0 : LS0 + TS]

        # ---- GPSIMD warm-up: absorb Q7 library load before real scatters
        # (all indices -1 -> pure zero-fill of a throwaway tile) ----
        dmy = sp.tile([P, 2], I16)
        nc.vector.memset(dmy[:], -1)
        warm = sp.tile([P, 2], BF16)
        nc.gpsimd.local_scatter(
            out_ap=warm[:], data_ap=dmy[:].bitcast(BF16), idxs_ap=dmy[:],
            channels=P, num_elems=2, num_idxs=2,
        )

        # ---- duplicate-group sums via prefix scan (f32 state) ----
        gs = sp.tile([P, TS], F32)
        nc.vector.tensor_tensor_scan(
            gs[:], ct, at, 0.0,
            op0=mybir.AluOpType.mult, op1=mybir.AluOpType.add,
        )

        # ---- p_gen = sigmoid(x @ w + b) via PE (fp16 in, f32 accum) ----
        d64 = pp.tile([BSH, 1], F32, space="PSUM")
        for k in range(NK):
            nc.tensor.matmul(
                d64[:],
                lhsT=xt[:, k * BSH : (k + 1) * BSH],
                rhs=wt[:, k : k + 1],
                start=(k == 0),
                stop=(k == NK - 1),
            )
        d64s = sp.tile([BSH, 1], F16)
        nc.vector.tensor_scalar_mul(d64s[:], d64[:], 1.0)
        dots = pp.tile([P, 1], F32, space="PSUM")
        nc.tensor.matmul(dots[:], lhsT=dupt[:], rhs=d64s[:], start=True, stop=True)
        pgd = sp.tile([P, 1], F32)
        nc.scalar.activation(
            pgd[:], dots[:], mybir.ActivationFunctionType.Sigmoid, bias=b_const
        )
        omd = sp.tile([P, 1], F32)  # 1 - p_gen
        nc.vector.tensor_scalar(
            omd[:], pgd[:], -1.0, 1.0,
            mybir.AluOpType.mult, mybir.AluOpType.add,
        )
        gsc = sp.tile([P, TS], BF16)  # (1 - p_gen) * group sums
        nc.scalar.mul(gsc[:], gs[:], omd[:])

        # ---- stream: out = p_gen * vocab_dist + sparse (bf16 out) ----
        for c in range(NSTREAM):
            tl = tls[c]
            spt = spp.tile([P, SW], BF16, tag="spt")
            nc.gpsimd.local_scatter(
                out_ap=spt[:],
                data_ap=gsc[:, c * S : (c + 1) * S],
                idxs_ap=lsi[:, c * S : (c + 1) * S],
                channels=P, num_elems=SW, num_idxs=S,
            )
            # Q7 write-visibility guard: a second serialized GPSIMD op that
            # overlaps the consumer's read range; its completion implies the
            # scatter's stores have landed (identity copy, so even a stale
            # read of its own two columns is harmless).
            nc.gpsimd.tensor_copy(out=spt[:, :2], in_=spt[:, :2])
            # tlb = (tl * p_gen) + sparse, fused on DVE, bf16 out
            tlb = spp.tile([P, SW], BF16, tag="tlb")
            nc.vector.scalar_tensor_tensor(
                tlb[:], tl[:], pgd[:], spt[:],
                op0=mybir.AluOpType.mult, op1=mybir.AluOpType.add,
            )
            nc.scalar.dma_start(outv[:, c * SW : (c + 1) * SW], tlb[:])


# --------------------------------------------------------------------------
# entry point
# --------------------------------------------------------------------------

last_results = None  # BassKernelResults of the most recent run (for benchmarks)


def build_program(b_const: float, S: int):
    TS = NB * S
    nc = bacc.Bacc("TRN2", target_bir_lowering=False, debug=False,
                   num_devices=NCORES)
    vd_t = nc.dram_tensor("vd", [BSH * V], F32, kind="ExternalInput")
    xw_t = nc.dram_tensor("xwside", [P, XW + NK], F16, kind="ExternalInput")
    sc_t = nc.dram_tensor("scside", [P, 3 * TS], I16, kind="ExternalInput")
    dup_t = nc.dram_tensor("dup", [BSH, P], F16, kind="ExternalInput")
    out_t = nc.dram_tensor("out", [BSH * V], BF16, kind="ExternalOutput")

    with tile.TileContext(nc) as tc:
        _build_kernel(
            tc,
            out_t.ap(),
            (vd_t.ap(), xw_t.ap(), sc_t.ap(), dup_t.ap()),
            b_const,
            S,
        )
    nc.compile()
    return nc


def prepare_in_maps(vocab_dist, attn_dist, xcat_full, wall_np, src_ids, vs, S):
    # wall laid out [P, NK]: wall[p, k] = w[k*128 + p]
    wall_t = np.ascontiguousarray(wall_np.reshape(NK, P).T).astype(np.float16)
    # duplication matrix: row b feeds partitions 2b and 2b+1
    dup = np.zeros((BSH, P), np.float16)
    dup[np.arange(BSH), 2 * np.arange(BSH)] = 1.0
    dup[np.arange(BSH), 2 * np.arange(BSH) + 1] = 1.0
    in_maps = []
    for c in range(NCORES):
        sl = slice(c * BSH, (c + 1) * BSH)
        attn_p, cont, lsidx = _prep_shard(attn_dist[sl], src_ids[sl], vs, S)
        # xT laid out [P, NK*BSH]: xT[p, k*BSH + m] = x[m, k*128 + p]
        xT = np.ascontiguousarray(
            xcat_full[sl].T.reshape(NK, P, BSH).transpose(1, 0, 2).reshape(P, -1)
        ).astype(np.float16)
        xwside = np.concatenate([xT, wall_t], axis=1)
        scside = np.concatenate(
            [attn_p.view(np.int16), cont.view(np.int16), lsidx], axis=1
        )
        in_maps.append(
            {
                "vd": np.ascontiguousarray(vocab_dist[sl]).reshape(-1),
                "xwside": np.ascontiguousarray(xwside),
                "scside": np.ascontiguousarray(scside),
                "dup": dup,
            }
        )
    return in_maps


def kernel(vocab_dist, attn_dist, context, state, emb, src_ids, vocab_size,
           w_c, w_s, w_y, b, **kwargs):
    vocab_dist = np.ascontiguousarray(np.asarray(vocab_dist, dtype=np.float32))
    attn_dist = np.asarray(attn_dist, dtype=np.float32)
    xcat_full = np.ascontiguousarray(
        np.concatenate(
            [np.asarray(context), np.asarray(state), np.asarray(emb)], axis=1
        ).astype(np.float32)
    )
    src_ids = np.asarray(src_ids)
    vs = int(np.asarray(vocab_size))
    wall_np = np.ascontiguousarray(
        np.concatenate(
            [np.asarray(w_c), np.asarray(w_s), np.asarray(w_y)]
        ).astype(np.float32)
    )
    b_const = float(np.asarray(b).reshape(-1)[0])

    assert vocab_dist.shape == (B, V) and attn_dist.shape == (B, T)
    assert xcat_full.shape == (B, D) and src_ids.shape == (B, T)

    S = _slot_requirement(src_ids, vs)
    nc = build_program(b_const, S)
    in_maps = prepare_in_maps(
        vocab_dist, attn_dist, xcat_full, wall_np, src_ids, vs, S
    )

    _trace = os.environ.get("PG_KERNEL_TRACE", "0") == "1"
    res = bass_utils.run_bass_kernel_spmd(
        nc, in_maps, core_ids=list(range(NCORES)), trace=_trace
    )
    global last_results
    last_results = res

    out = np.empty((B, V), np.float32)
    for c in range(NCORES):
        out[c * BSH : (c + 1) * BSH] = (
            res.results[c]["out"].astype(np.float32).reshape(BSH, V)
        )
    return out
